# revision 12
# baseline (speedup 1.0000x reference)
"""MiniS4D Trainium2 kernel — 8 NeuronCores, channel-sharded SSM phase +
AllToAll exchange + batch-sharded channel-mix phase.

Phase 1 (per core: 64 channels, ALL 16 batches): chunked S4D conv
(T=128, M=32 chunks) — per-channel Toeplitz matmul for intra-chunk lags
(+ D folded into lag 0) and a prefix-scan state-space path for
inter-chunk lags, exactly the baseline decomposition but with 512-col
matmuls (16 batches x 32 chunks) instead of 64-col.

Exchange: GELU(y) for (batch-pair bb, channel-half) blocks is staged
c-contiguous and AllToAll'd so core d ends with y[2d:2d+2, all 512 c, L].

Phase 2 (per core: its 2 batches): PE-transpose received y to
channel-major, W_out matmul, GLU, mean, decoder. Output (1,2) per core.
"""
import sys, os
sys.path.insert(0, "/opt/trn_rl_repo")
import numpy as np

import concourse.bass as bass
import concourse.tile as tile
from concourse import bacc, mybir
from concourse import bass_utils

F32 = mybir.dt.float32
F16 = mybir.dt.float16
F8 = mybir.dt.float8e4
AF = mybir.ActivationFunctionType
ALU = mybir.AluOpType

B, C, L, N = 16, 512, 4096, 8
T, M = 128, 32
NCORES, CL = 8, 64          # channels per core
SLOTS = 32
RG = [[0, 1, 2, 3, 4, 5, 6, 7]]

Y8 = False                   # fp8 exchange + DoubleRow mix
YD = F8 if Y8 else F16

_compiled = None


def _prep(inputs):
    """Host-side parameter preparation. Returns list of per-core in_maps."""
    log_dt = inputs["log_dt"].astype(np.float64)
    A = -np.exp(inputs["log_A_real"].astype(np.float64)) \
        + 1j * inputs["A_imag"].astype(np.float64)
    dt = np.exp(log_dt)
    r = np.exp(dt[:, None] * A)                                   # (C, N)
    Bc = inputs["B_re"].astype(np.float64) + 1j * inputs["B_im"].astype(np.float64)
    Cc = inputs["C_re"].astype(np.float64) + 1j * inputs["C_im"].astype(np.float64)
    w = Cc * (r - 1.0) / A * Bc                                   # (C, N)
    rinv = 1.0 / r
    wL = w * r ** (L - 1)

    lags = np.arange(T)
    keff = np.real(wL[:, :, None] * rinv[:, :, None] ** lags).sum(1)  # (C, T)
    keff[:, 0] += inputs["D"].astype(np.float64)

    toep = np.zeros((C, T, T), np.float16)
    for d in range(T):
        idx = np.arange(T - d)
        toep[:, idx, idx + d] = keff[:, d].astype(np.float16)[:, None]

    pw = r[:, :, None] ** lags                                     # (C, N, T)
    v2 = np.zeros((T, C, 32), np.float16)                          # [t, c, 2n pad]
    v2[:, :, 0:8] = pw.real.transpose(2, 0, 1)
    v2[:, :, 8:16] = pw.imag.transpose(2, 0, 1)

    pw1 = w[:, :, None] * rinv[:, :, None] ** lags                 # (C, N, T)
    v1rows = np.concatenate([pw1.real, -pw1.imag], 1)              # (C, 16, T)

    s_idx = np.arange(SLOTS)
    e = r ** T
    Epow = e[:, :, None] ** np.maximum(s_idx - 1, 0)               # (C, N, S)
    Epow[:, :, 0] = 0.0
    Kpow = r[:, :, None] ** (L - 1 - T * s_idx)                    # (C, N, S)
    Kpow[:, :, 0] = 0.0

    # per-core scale tiles [128, 16, S]: p = 32q + ss; channel cl = 4*cg2 + q
    def scale_tiles(Z):
        t1 = np.zeros((NCORES, 128, 16, SLOTS), np.float16)
        t2 = np.zeros((NCORES, 128, 16, SLOTS), np.float16)
        cg2 = np.arange(16)
        for k in range(NCORES):
            for q in range(4):
                cs = 64 * k + 4 * cg2 + q                           # (16,)
                for ss in range(16):
                    p = 32 * q + ss
                    n = ss % 8
                    sgn = -1.0 if ss < 8 else 1.0
                    t1[k, p] = Z[cs, n, :].real.astype(np.float16)
                    t2[k, p] = (sgn * Z[cs, n, :].imag).astype(np.float16)
        return t1, t2

    e1a, e2a = scale_tiles(Epow)
    k1a, k2a = scale_tiles(Kpow)

    # v1 lhsT per core: [128, 16, 2, T]; q<3 at rows 32q col 0,
    # q=3 mirrored to rows 0:16 col 1 (PE quadrant-3 weight loads invalid)
    v1a = np.zeros((NCORES, 128, 16, 2, T), np.float16)
    for k in range(NCORES):
        for cg2 in range(16):
            for q in range(4):
                row, col = (32 * q, 0) if q < 3 else (0, 1)
                v1a[k, row:row + 16, cg2, col, :] = \
                    v1rows[64 * k + 4 * cg2 + q].astype(np.float16)

    # mix weights [4 ctt, 128 p, 1024 o]; ctt = 2*chalf + sh;
    # p -> global channel 64*(4*sh + p//32) + 32*chalf + p%32
    ydt_np = mybir.dt.np(YD)
    W = inputs["W_out"].astype(np.float32)                          # (1024, 512)
    wmixh = np.zeros((4, 128, 1024), np.float32)
    for ctt in range(4):
        chalf, sh = ctt // 2, ctt % 2
        p = np.arange(128)
        gc = 64 * (4 * sh + p // 32) + 32 * chalf + p % 32
        wmixh[ctt] = W[:, gc].T
    wmixh = wmixh.astype(ydt_np)
    eyeh = np.eye(128, dtype=np.float32).astype(ydt_np)

    b_out = inputs["b_out"].astype(np.float32)
    bouta = np.ascontiguousarray(b_out[:512].reshape(4, 128).T)     # (128, 4)
    boutg = np.ascontiguousarray(b_out[512:].reshape(4, 128).T)
    wd = (inputs["W_dec"][0].astype(np.float32) / L).reshape(4, 128).T
    wdech = np.ascontiguousarray(np.repeat(wd[:, None, :], 2, axis=1))  # (128,2,4)
    bdech = inputs["b_dec"].astype(np.float32).reshape(1, 1)

    u16 = inputs["u"].astype(np.float16)                            # (B, C, L)

    in_maps = []
    for k in range(NCORES):
        cs = slice(64 * k, 64 * k + 64)
        in_maps.append(dict(
            u16=np.ascontiguousarray(u16[:, cs, :]),
            toeph=np.ascontiguousarray(toep[cs]),
            v2h=np.ascontiguousarray(v2[:, cs, :]),
            v1h=v1a[k],
            e1h=e1a[k], e2h=e2a[k], k1h=k1a[k], k2h=k2a[k],
            wmixh=wmixh, eyeh=eyeh,
            bouta=bouta, boutg=boutg, wdech=wdech, bdech=bdech,
        ))
    return in_maps


def _build():
    nc = bacc.Bacc("TRN2", target_bir_lowering=False, debug=False,
                   num_devices=NCORES)
    d_u = nc.dram_tensor("u16", [B, CL, L], F16, kind="ExternalInput").ap()
    d_toep = nc.dram_tensor("toeph", [CL, T, T], F16, kind="ExternalInput").ap()
    d_v2 = nc.dram_tensor("v2h", [T, CL, 32], F16, kind="ExternalInput").ap()
    d_v1 = nc.dram_tensor("v1h", [128, 16, 2, T], F16, kind="ExternalInput").ap()
    d_e1 = nc.dram_tensor("e1h", [128, 16, SLOTS], F16, kind="ExternalInput").ap()
    d_e2 = nc.dram_tensor("e2h", [128, 16, SLOTS], F16, kind="ExternalInput").ap()
    d_k1 = nc.dram_tensor("k1h", [128, 16, SLOTS], F16, kind="ExternalInput").ap()
    d_k2 = nc.dram_tensor("k2h", [128, 16, SLOTS], F16, kind="ExternalInput").ap()
    d_wmix = nc.dram_tensor("wmixh", [4, 128, 1024], YD, kind="ExternalInput").ap()
    d_eye = nc.dram_tensor("eyeh", [128, 128], YD, kind="ExternalInput").ap()
    d_bouta = nc.dram_tensor("bouta", [128, 4], F32, kind="ExternalInput").ap()
    d_boutg = nc.dram_tensor("boutg", [128, 4], F32, kind="ExternalInput").ap()
    d_wdec = nc.dram_tensor("wdech", [128, 2, 4], F32, kind="ExternalInput").ap()
    d_bdec = nc.dram_tensor("bdech", [1, 1], F32, kind="ExternalInput").ap()
    d_out = nc.dram_tensor("odec", [1, 2], F32, kind="ExternalOutput").ap()

    SHUF = [(i + 8) % 16 if i < 16 else i for i in range(32)]

    with tile.TileContext(nc) as tc:
        with tc.tile_pool(name="dram", bufs=1, space="DRAM") as dram:
            # exchange blocks: blk = 2*bb + chalf, layout [8 d, 128 j, 32 m, 32 c]
            ysend = [dram.tile([8, T, M, 32], YD, name=f"ysend{i}")
                     for i in range(4)]
            yrecv = [dram.tile([8, T, M, 32], YD, name=f"yrecv{i}")
                     for i in range(4)]

            biasp_cm = tc.tile_pool(name="biasp", bufs=1)
            biasp = biasp_cm.__enter__()
            bouta_sb = biasp.tile([128, 4], F32)
            nc.scalar.dma_start(bouta_sb[:], d_bouta[:])
            boutg_sb = biasp.tile([128, 4], F32)
            nc.scalar.dma_start(boutg_sb[:], d_boutg[:])
            wdec_sb = biasp.tile([128, 2, 4], F32)
            nc.scalar.dma_start(wdec_sb[:], d_wdec[:])
            bdec_sb = biasp.tile([1, 1], F32)
            nc.scalar.dma_start(bdec_sb[:], d_bdec[:])

            with tc.tile_pool(name="const", bufs=1) as constp:
                toepsb = constp.tile([T, CL, T], F16)
                nc.scalar.dma_start(toepsb[:], d_toep.transpose([1, 0, 2]))
                v2sb = constp.tile([T, CL, 32], F16)
                nc.scalar.dma_start(v2sb[:], d_v2[:])
                v1sb = constp.tile([128, 16, 2, T], F16)
                nc.scalar.dma_start(v1sb[:], d_v1[:])
                e1sb = constp.tile([128, 16, SLOTS], F16)
                nc.scalar.dma_start(e1sb[:], d_e1[:])
                e2sb = constp.tile([128, 16, SLOTS], F16)
                nc.scalar.dma_start(e2sb[:], d_e2[:])
                k1sb = constp.tile([128, 16, SLOTS], F16)
                nc.scalar.dma_start(k1sb[:], d_k1[:])
                k2sb = constp.tile([128, 16, SLOTS], F16)
                nc.scalar.dma_start(k2sb[:], d_k2[:])
                # ============== Phase 1: SSM =============================
                with tc.tile_pool(name="uTp", bufs=1) as uTp, \
                     tc.tile_pool(name="Hp", bufs=1) as Hp:
                    uT = uTp.tile([T, B, M, CL], F16)      # [t, b, m, cl]
                    for b in range(B):
                        nc.sync.dma_start_transpose(uT[:, b], d_u[b])

                    H = Hp.tile([128, 16, B, SLOTS], F16)  # [32q+s, cg2, b, sl]
                    H96 = Hp.tile([32, 16, B, SLOTS], F16)
                    nc.vector.memset(H[:, :, :, 0:1], 0.0)
                    nc.vector.memset(H96[:, :, :, 0:1], 0.0)

                    # ---- B1: V2 matmuls + prescale, then scan ----
                    with tc.tile_pool(name="hps", bufs=2, space="PSUM") as hps, \
                         tc.tile_pool(name="sclp", bufs=2) as sclp:
                        for cg2 in range(16):
                            hb = hps.tile([128, B, M], F32)
                            for q in range(4):
                                cl = 4 * cg2 + q
                                nc.tensor.matmul(
                                    hb[32 * q:32 * q + 32],
                                    v2sb[:, cl:cl + 1, :].squeeze(),
                                    uT[:, :, :, cl:cl + 1].squeeze(),
                                    start=True, stop=True,
                                    tile_position=(0, 32 * q))
                            sw = sclp.tile([128, B, SLOTS - 1], F32, tag="sw")
                            nc.vector.stream_shuffle(sw[:], hb[:, :, 0:31], SHUF)
                            t1 = sclp.tile([128, B, SLOTS - 1], F16, tag="t1")
                            nc.vector.tensor_mul(
                                t1[:], hb[:, :, 0:31],
                                e1sb[:, cg2:cg2 + 1, 1:32]
                                .broadcast_to((128, B, 31)))
                            nc.vector.tensor_mul(
                                sw[:], sw[:],
                                e2sb[:, cg2:cg2 + 1, 1:32]
                                .broadcast_to((128, B, 31)))
                            nc.vector.tensor_add(
                                H[:, cg2:cg2 + 1, :, 1:32].squeeze(),
                                t1[:], sw[:])
                        # prefix scan over slots (chunks)
                        for sl in range(2, SLOTS):
                            nc.vector.tensor_add(
                                H[:, :, :, sl:sl + 1],
                                H[:, :, :, sl:sl + 1],
                                H[:, :, :, sl - 1:sl])

                    # ---- B2: postscale/cg2 + Toeplitz + V1 + GELU + send ----
                    with tc.tile_pool(name="yps", bufs=4, space="PSUM") as ypsp, \
                         tc.tile_pool(name="stp", bufs=1) as stp, \
                         tc.tile_pool(name="ps2", bufs=2) as ps2:
                        st = {}
                        for cg2 in range(16):
                            # postscale cg2: H = k1*H + k2*shuffle(H)
                            hgc = H[:, cg2:cg2 + 1, :, 1:32].squeeze()
                            sw2 = ps2.tile([128, B, 31], F16, tag="sw2")
                            t2 = ps2.tile([128, B, 31], F16, tag="t2")
                            nc.vector.stream_shuffle(sw2[:], hgc, SHUF)
                            nc.vector.tensor_mul(
                                t2[:], hgc,
                                k1sb[:, cg2:cg2 + 1, 1:32]
                                .broadcast_to((128, B, 31)))
                            nc.vector.tensor_mul(
                                sw2[:], sw2[:],
                                k2sb[:, cg2:cg2 + 1, 1:32]
                                .broadcast_to((128, B, 31)))
                            nc.vector.tensor_add(hgc, t2[:], sw2[:])
                            nc.vector.tensor_copy(
                                H96[0:16, cg2:cg2 + 1, :, 1:32],
                                H[96:112, cg2:cg2 + 1, :, 1:32])

                            if cg2 % 8 == 0:
                                st[cg2 // 8] = stp.tile([T, 8, 2, M, 32], YD,
                                                        name=f"st{cg2 // 8}")
                            for q in range(4):
                                cl = 4 * cg2 + q
                                chalf, c32 = cl // 32, cl % 32
                                yb = ypsp.tile([128, B, M], F32)
                                nc.tensor.matmul(
                                    yb[:],
                                    toepsb[:, cl:cl + 1, :].squeeze(),
                                    uT[:, :, :, cl:cl + 1].squeeze(),
                                    start=True, stop=False)
                                if q < 3:
                                    v1_l = v1sb[32 * q:32 * q + 16,
                                                cg2:cg2 + 1, 0:1, :].squeeze()
                                    h_r = H[32 * q:32 * q + 16,
                                            cg2:cg2 + 1, :, :].squeeze()
                                else:
                                    v1_l = v1sb[0:16, cg2:cg2 + 1,
                                                1:2, :].squeeze()
                                    h_r = H96[0:16, cg2:cg2 + 1, :, :].squeeze()
                                nc.tensor.matmul(
                                    yb[:], v1_l, h_r,
                                    start=False, stop=True)
                                nc.scalar.activation(
                                    st[chalf][:, :, :, :, c32:c32 + 1].squeeze(),
                                    yb[:], AF.Gelu)
                            if cg2 == 7:
                                nc.sync.dma_start(
                                    ysend[0][:].transpose([1, 0, 2, 3]),
                                    st[0][:, :, 0:1, :, :].squeeze())
                                nc.gpsimd.collective_compute(
                                    "AllToAll", ALU.bypass, replica_groups=RG,
                                    ins=[ysend[0].opt()], outs=[yrecv[0].opt()])
                            if cg2 == 15:
                                nc.sync.dma_start(
                                    ysend[1][:].transpose([1, 0, 2, 3]),
                                    st[1][:, :, 0:1, :, :].squeeze())
                                nc.gpsimd.collective_compute(
                                    "AllToAll", ALU.bypass, replica_groups=RG,
                                    ins=[ysend[1].opt()], outs=[yrecv[1].opt()])
                                nc.sync.dma_start(
                                    ysend[2][:].transpose([1, 0, 2, 3]),
                                    st[0][:, :, 1:2, :, :].squeeze())
                                nc.gpsimd.collective_compute(
                                    "AllToAll", ALU.bypass, replica_groups=RG,
                                    ins=[ysend[2].opt()], outs=[yrecv[2].opt()])
                                nc.sync.dma_start(
                                    ysend[3][:].transpose([1, 0, 2, 3]),
                                    st[1][:, :, 1:2, :, :].squeeze())
                                nc.gpsimd.collective_compute(
                                    "AllToAll", ALU.bypass, replica_groups=RG,
                                    ins=[ysend[3].opt()], outs=[yrecv[3].opt()])

            # ============== Phase 2: mix ================================
            with tc.tile_pool(name="ytp", bufs=1) as ytp, \
                 tc.tile_pool(name="ytjp", bufs=2) as ytjp, \
                 tc.tile_pool(name="tpp", bufs=2, space="PSUM") as tpp, \
                 tc.tile_pool(name="zps", bufs=2, space="PSUM") as zpsp, \
                 tc.tile_pool(name="mxs", bufs=4) as mxsp, \
                 tc.tile_pool(name="m1p", bufs=1) as m1p:
                wm = m1p.tile([128, 4, 1024], YD)
                nc.scalar.dma_start(wm[:], d_wmix.transpose([1, 0, 2]))
                idsb = m1p.tile([128, 128], YD)
                nc.scalar.dma_start(idsb[:], d_eye[:])
                M1 = m1p.tile([128, 2, 4, 8], F32)
                yt = {}
                for bb in range(2):
                    yt[bb] = ytp.tile([128, 4, T, M], YD, name=f"yt{bb}")
                    for ctt in range(4):
                        chalf, sh = ctt // 2, ctt % 2
                        ytj = ytjp.tile([T, M, 128], YD)
                        for si in range(4):
                            s = 4 * sh + si
                            eng = nc.scalar if bb == 0 else nc.sync
                            eng.dma_start(
                                ytj[:, :, 32 * si:32 * si + 32],
                                yrecv[2 * bb + chalf][s:s + 1].squeeze())
                        for g in range(8):
                            tp = tpp.tile([128, 4, T], YD)
                            for i in range(4):
                                m = 4 * g + i
                                nc.tensor.transpose(
                                    tp[:, i:i + 1, :].squeeze(),
                                    ytj[:, m:m + 1, :].squeeze(),
                                    idsb[:])
                            nc.vector.tensor_copy(
                                yt[bb][:, ctt:ctt + 1, :, 4 * g:4 * g + 4]
                                .squeeze(),
                                tp[:].transpose([0, 2, 1]))

                    for pr in range(4):
                        for lg in range(8):
                            zg = zpsp.tile([128, 16, M], F32)
                            za = zpsp.tile([128, 16, M], F32)
                            for side, ztile in ((1, zg), (0, za)):
                                ob = pr + 4 * side
                                if Y8:
                                    for t in range(2):
                                        nc.tensor.matmul(
                                            ztile[:],
                                            wm[:, 2 * t:2 * t + 2,
                                               128 * ob:128 * ob + 128],
                                            yt[bb][:, 2 * t:2 * t + 2,
                                                   16 * lg:16 * lg + 16, :],
                                            start=(t == 0), stop=(t == 1),
                                            perf_mode=mybir.MatmulPerfMode
                                            .DoubleRow)
                                else:
                                    for ctt in range(4):
                                        nc.tensor.matmul(
                                            ztile[:],
                                            wm[:, ctt:ctt + 1,
                                               128 * ob:128 * ob + 128]
                                            .squeeze(),
                                            yt[bb][:, ctt:ctt + 1,
                                                   16 * lg:16 * lg + 16, :]
                                            .squeeze(),
                                            start=(ctt == 0), stop=(ctt == 3))
                            sgm = mxsp.tile([128, 16, M], F16, tag="sgm")
                            nc.scalar.activation(
                                sgm[:], zg[:], AF.Sigmoid,
                                bias=boutg_sb[:, pr:pr + 1])
                            scr = mxsp.tile([128, 16, M], F16, tag="scr")
                            nc.vector.scalar_tensor_tensor(
                                scr[:], za[:], bouta_sb[:, pr:pr + 1], sgm[:],
                                op0=ALU.add, op1=ALU.mult,
                                accum_out=M1[:, bb:bb + 1, pr:pr + 1,
                                             lg:lg + 1].squeeze().unsqueeze(1))

                # ---- decode ----
                with tc.tile_pool(name="dps", bufs=1, space="PSUM") as dpsp:
                    R1 = m1p.tile([128, 2, 4], F32)
                    nc.vector.reduce_sum(R1[:], M1[:], axis=mybir.AxisListType.X)
                    R2 = m1p.tile([128, 2, 4], F32)
                    nc.vector.tensor_mul(R2[:], R1[:], wdec_sb[:])
                    R3 = m1p.tile([128, 2], F32)
                    nc.vector.reduce_sum(R3[:], R2[:], axis=mybir.AxisListType.X)
                    ones = m1p.tile([128, 1], F32)
                    nc.vector.memset(ones[:], 1.0)
                    dp = dpsp.tile([1, 2], F32)
                    nc.tensor.matmul(dp[:], ones[:], R3[:], start=True, stop=True)
                    osb = m1p.tile([1, 2], F32)
                    nc.vector.tensor_scalar_add(osb[:], dp[:], bdec_sb[:, 0:1])
                    nc.sync.dma_start(d_out[:], osb[:])

            biasp_cm.__exit__(None, None, None)

    nc.compile()
    return nc


def _get_compiled():
    global _compiled
    if _compiled is None:
        _compiled = _build()
    return _compiled


def _run(inputs, trace=False, **kw):
    in_maps = _prep(inputs)
    nc = _get_compiled()
    return bass_utils.run_bass_kernel_spmd(
        nc, in_maps, core_ids=list(range(NCORES)), trace=trace, **kw)


def kernel(**inputs):
    inputs = {k: np.asarray(v) for k, v in inputs.items()}
    res = _run(inputs)
    out = np.empty((B, 1), np.float32)
    for cid in range(NCORES):
        out[2 * cid:2 * cid + 2, 0] = res.results[cid]["odec"][0, :]
    return out


# revision 13
# speedup vs baseline: 1.0724x; 1.0724x over previous
"""MiniS4D Trainium2 kernel — 8 NeuronCores, channel-sharded SSM phase +
AllToAll exchange + batch-sharded channel-mix phase.

Phase 1 (per core: 64 channels, ALL 16 batches): chunked S4D conv
(T=128, M=32 chunks) — per-channel Toeplitz matmul for intra-chunk lags
(+ D folded into lag 0) and a prefix-scan state-space path for
inter-chunk lags, exactly the baseline decomposition but with 512-col
matmuls (16 batches x 32 chunks) instead of 64-col.

Exchange: GELU(y) for (batch-pair bb, channel-half) blocks is staged
c-contiguous and AllToAll'd so core d ends with y[2d:2d+2, all 512 c, L].

Phase 2 (per core: its 2 batches): PE-transpose received y to
channel-major, W_out matmul, GLU, mean, decoder. Output (1,2) per core.
"""
import sys, os
sys.path.insert(0, "/opt/trn_rl_repo")
import numpy as np

import concourse.bass as bass
import concourse.tile as tile
from concourse import bacc, mybir
from concourse import bass_utils

F32 = mybir.dt.float32
F16 = mybir.dt.float16
F8 = mybir.dt.float8e4
AF = mybir.ActivationFunctionType
ALU = mybir.AluOpType

B, C, L, N = 16, 512, 4096, 8
T, M = 128, 32
NCORES, CL = 8, 64          # channels per core
SLOTS = 32
RG = [[0, 1, 2, 3, 4, 5, 6, 7]]

Y8 = False                   # fp8 exchange + DoubleRow mix
YD = F8 if Y8 else F16

_compiled = None


def _prep(inputs):
    """Host-side parameter preparation. Returns list of per-core in_maps."""
    log_dt = inputs["log_dt"].astype(np.float64)
    A = -np.exp(inputs["log_A_real"].astype(np.float64)) \
        + 1j * inputs["A_imag"].astype(np.float64)
    dt = np.exp(log_dt)
    r = np.exp(dt[:, None] * A)                                   # (C, N)
    Bc = inputs["B_re"].astype(np.float64) + 1j * inputs["B_im"].astype(np.float64)
    Cc = inputs["C_re"].astype(np.float64) + 1j * inputs["C_im"].astype(np.float64)
    w = Cc * (r - 1.0) / A * Bc                                   # (C, N)
    rinv = 1.0 / r
    wL = w * r ** (L - 1)

    lags = np.arange(T)
    keff = np.real(wL[:, :, None] * rinv[:, :, None] ** lags).sum(1)  # (C, T)
    keff[:, 0] += inputs["D"].astype(np.float64)

    toep = np.zeros((C, T, T), np.float16)
    for d in range(T):
        idx = np.arange(T - d)
        toep[:, idx, idx + d] = keff[:, d].astype(np.float16)[:, None]

    pw = r[:, :, None] ** lags                                     # (C, N, T)
    v2 = np.zeros((T, C, 32), np.float16)                          # [t, c, 2n pad]
    v2[:, :, 0:8] = pw.real.transpose(2, 0, 1)
    v2[:, :, 8:16] = pw.imag.transpose(2, 0, 1)

    pw1 = w[:, :, None] * rinv[:, :, None] ** lags                 # (C, N, T)
    v1rows = np.concatenate([pw1.real, -pw1.imag], 1)              # (C, 16, T)

    s_idx = np.arange(SLOTS)
    e = r ** T
    Epow = e[:, :, None] ** np.maximum(s_idx - 1, 0)               # (C, N, S)
    Epow[:, :, 0] = 0.0
    Kpow = r[:, :, None] ** (L - 1 - T * s_idx)                    # (C, N, S)
    Kpow[:, :, 0] = 0.0

    # per-core scale tiles [128, 16, S]: p = 32q + ss; channel cl = 4*cg2 + q
    def scale_tiles(Z):
        t1 = np.zeros((NCORES, 128, 16, SLOTS), np.float16)
        t2 = np.zeros((NCORES, 128, 16, SLOTS), np.float16)
        cg2 = np.arange(16)
        for k in range(NCORES):
            for q in range(4):
                cs = 64 * k + 4 * cg2 + q                           # (16,)
                for ss in range(16):
                    p = 32 * q + ss
                    n = ss % 8
                    sgn = -1.0 if ss < 8 else 1.0
                    t1[k, p] = Z[cs, n, :].real.astype(np.float16)
                    t2[k, p] = (sgn * Z[cs, n, :].imag).astype(np.float16)
        return t1, t2

    e1a, e2a = scale_tiles(Epow)
    k1a, k2a = scale_tiles(Kpow)

    # v1 lhsT per core: [128, 16, 2, T]; q<3 at rows 32q col 0,
    # q=3 mirrored to rows 0:16 col 1 (PE quadrant-3 weight loads invalid)
    v1a = np.zeros((NCORES, 128, 16, 2, T), np.float16)
    for k in range(NCORES):
        for cg2 in range(16):
            for q in range(4):
                row, col = (32 * q, 0) if q < 3 else (0, 1)
                v1a[k, row:row + 16, cg2, col, :] = \
                    v1rows[64 * k + 4 * cg2 + q].astype(np.float16)

    # mix weights [4 ctt, 128 p, 1024 o]; ctt = 2*chalf + sh;
    # p -> global channel 64*(4*sh + p//32) + 32*chalf + p%32
    ydt_np = mybir.dt.np(YD)
    W = inputs["W_out"].astype(np.float32)                          # (1024, 512)
    wmixh = np.zeros((4, 128, 1024), np.float32)
    for ctt in range(4):
        chalf, sh = ctt // 2, ctt % 2
        p = np.arange(128)
        gc = 64 * (4 * sh + p // 32) + 32 * chalf + p % 32
        wmixh[ctt] = W[:, gc].T
    wmixh = wmixh.astype(ydt_np)

    b_out = inputs["b_out"].astype(np.float32)
    bouta = np.ascontiguousarray(b_out[:512].reshape(4, 128).T)     # (128, 4)
    boutg = np.ascontiguousarray(b_out[512:].reshape(4, 128).T)
    wd = (inputs["W_dec"][0].astype(np.float32) / L).reshape(4, 128).T
    wdech = np.ascontiguousarray(np.repeat(wd[:, None, :], 2, axis=1))  # (128,2,4)
    bdech = inputs["b_dec"].astype(np.float32).reshape(1, 1)

    u16 = inputs["u"].astype(np.float16)                            # (B, C, L)

    in_maps = []
    for k in range(NCORES):
        cs = slice(64 * k, 64 * k + 64)
        in_maps.append(dict(
            u16=np.ascontiguousarray(u16[:, cs, :]),
            toeph=np.ascontiguousarray(toep[cs]),
            v2h=np.ascontiguousarray(v2[:, cs, :]),
            v1h=v1a[k],
            e1h=e1a[k], e2h=e2a[k], k1h=k1a[k], k2h=k2a[k],
            wmixh=wmixh,
            bouta=bouta, boutg=boutg, wdech=wdech, bdech=bdech,
        ))
    return in_maps


def _build():
    nc = bacc.Bacc("TRN2", target_bir_lowering=False, debug=False,
                   num_devices=NCORES)
    d_u = nc.dram_tensor("u16", [B, CL, L], F16, kind="ExternalInput").ap()
    d_toep = nc.dram_tensor("toeph", [CL, T, T], F16, kind="ExternalInput").ap()
    d_v2 = nc.dram_tensor("v2h", [T, CL, 32], F16, kind="ExternalInput").ap()
    d_v1 = nc.dram_tensor("v1h", [128, 16, 2, T], F16, kind="ExternalInput").ap()
    d_e1 = nc.dram_tensor("e1h", [128, 16, SLOTS], F16, kind="ExternalInput").ap()
    d_e2 = nc.dram_tensor("e2h", [128, 16, SLOTS], F16, kind="ExternalInput").ap()
    d_k1 = nc.dram_tensor("k1h", [128, 16, SLOTS], F16, kind="ExternalInput").ap()
    d_k2 = nc.dram_tensor("k2h", [128, 16, SLOTS], F16, kind="ExternalInput").ap()
    d_wmix = nc.dram_tensor("wmixh", [4, 128, 1024], YD, kind="ExternalInput").ap()
    d_bouta = nc.dram_tensor("bouta", [128, 4], F32, kind="ExternalInput").ap()
    d_boutg = nc.dram_tensor("boutg", [128, 4], F32, kind="ExternalInput").ap()
    d_wdec = nc.dram_tensor("wdech", [128, 2, 4], F32, kind="ExternalInput").ap()
    d_bdec = nc.dram_tensor("bdech", [1, 1], F32, kind="ExternalInput").ap()
    d_out = nc.dram_tensor("odec", [1, 2], F32, kind="ExternalOutput").ap()

    SHUF = [(i + 8) % 16 if i < 16 else i for i in range(32)]

    with tile.TileContext(nc) as tc:
        with tc.tile_pool(name="dram", bufs=1, space="DRAM") as dram:
            # exchange blocks: blk = 2*bb + chalf, layout [8 d, 32 c, 128 j, 32 m]
            ysend = [dram.tile([8, 32, T, M], YD, name=f"ysend{i}")
                     for i in range(4)]
            yrecv = [dram.tile([8, 32, T, M], YD, name=f"yrecv{i}")
                     for i in range(4)]

            biasp_cm = tc.tile_pool(name="biasp", bufs=1)
            biasp = biasp_cm.__enter__()
            bouta_sb = biasp.tile([128, 4], F32)
            nc.scalar.dma_start(bouta_sb[:], d_bouta[:])
            boutg_sb = biasp.tile([128, 4], F32)
            nc.scalar.dma_start(boutg_sb[:], d_boutg[:])
            wdec_sb = biasp.tile([128, 2, 4], F32)
            nc.scalar.dma_start(wdec_sb[:], d_wdec[:])
            bdec_sb = biasp.tile([1, 1], F32)
            nc.scalar.dma_start(bdec_sb[:], d_bdec[:])

            with tc.tile_pool(name="const", bufs=1) as constp:
                toepsb = constp.tile([T, CL, T], F16)
                nc.scalar.dma_start(toepsb[:], d_toep.transpose([1, 0, 2]))
                v2sb = constp.tile([T, CL, 32], F16)
                nc.scalar.dma_start(v2sb[:], d_v2[:])
                v1sb = constp.tile([128, 16, 2, T], F16)
                nc.scalar.dma_start(v1sb[:], d_v1[:])
                e1sb = constp.tile([128, 16, SLOTS], F16)
                nc.scalar.dma_start(e1sb[:], d_e1[:])
                e2sb = constp.tile([128, 16, SLOTS], F16)
                nc.scalar.dma_start(e2sb[:], d_e2[:])
                k1sb = constp.tile([128, 16, SLOTS], F16)
                nc.scalar.dma_start(k1sb[:], d_k1[:])
                k2sb = constp.tile([128, 16, SLOTS], F16)
                nc.scalar.dma_start(k2sb[:], d_k2[:])
                # ============== Phase 1: SSM =============================
                with tc.tile_pool(name="uTp", bufs=1) as uTp, \
                     tc.tile_pool(name="Hp", bufs=1) as Hp:
                    uT = uTp.tile([T, B, M, CL], F16)      # [t, b, m, cl]
                    for b in range(B):
                        nc.sync.dma_start_transpose(uT[:, b], d_u[b])

                    H = Hp.tile([128, 16, B, SLOTS], F16)  # [32q+s, cg2, b, sl]
                    H96 = Hp.tile([32, 16, B, SLOTS], F16)
                    nc.vector.memset(H[:, :, :, 0:1], 0.0)
                    nc.vector.memset(H96[:, :, :, 0:1], 0.0)

                    # ---- B1: V2 matmuls + prescale, then scan ----
                    with tc.tile_pool(name="hps", bufs=2, space="PSUM") as hps, \
                         tc.tile_pool(name="sclp", bufs=2) as sclp:
                        for cg2 in range(16):
                            hb = hps.tile([128, B, M], F32)
                            for q in range(4):
                                cl = 4 * cg2 + q
                                nc.tensor.matmul(
                                    hb[32 * q:32 * q + 32],
                                    v2sb[:, cl:cl + 1, :].squeeze(),
                                    uT[:, :, :, cl:cl + 1].squeeze(),
                                    start=True, stop=True,
                                    tile_position=(0, 32 * q))
                            sw = sclp.tile([128, B, SLOTS - 1], F32, tag="sw")
                            nc.vector.stream_shuffle(sw[:], hb[:, :, 0:31], SHUF)
                            t1 = sclp.tile([128, B, SLOTS - 1], F16, tag="t1")
                            nc.vector.tensor_mul(
                                t1[:], hb[:, :, 0:31],
                                e1sb[:, cg2:cg2 + 1, 1:32]
                                .broadcast_to((128, B, 31)))
                            nc.vector.tensor_mul(
                                sw[:], sw[:],
                                e2sb[:, cg2:cg2 + 1, 1:32]
                                .broadcast_to((128, B, 31)))
                            nc.vector.tensor_add(
                                H[:, cg2:cg2 + 1, :, 1:32].squeeze(),
                                t1[:], sw[:])
                        # prefix scan over slots (chunks)
                        for sl in range(2, SLOTS):
                            nc.vector.tensor_add(
                                H[:, :, :, sl:sl + 1],
                                H[:, :, :, sl:sl + 1],
                                H[:, :, :, sl - 1:sl])

                    # ---- B2: postscale/cg2 + Toeplitz + V1 + GELU + send ----
                    with tc.tile_pool(name="yps", bufs=4, space="PSUM") as ypsp, \
                         tc.tile_pool(name="sgp", bufs=6) as sgp, \
                         tc.tile_pool(name="ps2", bufs=2) as ps2:
                        for cg2 in range(16):
                            if cg2 % 4 == 0:
                                # postscale 4 groups: H = k1*H + k2*shuffle(H)
                                hgc = H[:, cg2:cg2 + 4, :, 1:32]
                                sw2 = ps2.tile([128, 4, B, 31], F16, tag="sw2")
                                t2 = ps2.tile([128, 4, B, 31], F16, tag="t2")
                                nc.vector.stream_shuffle(sw2[:], hgc, SHUF)
                                nc.vector.tensor_mul(
                                    t2[:], hgc,
                                    k1sb[:, cg2:cg2 + 4, 1:32].unsqueeze(2)
                                    .broadcast_to((128, 4, B, 31)))
                                nc.vector.tensor_mul(
                                    sw2[:], sw2[:],
                                    k2sb[:, cg2:cg2 + 4, 1:32].unsqueeze(2)
                                    .broadcast_to((128, 4, B, 31)))
                                nc.vector.tensor_add(hgc, t2[:], sw2[:])
                                nc.vector.tensor_copy(
                                    H96[0:16, cg2:cg2 + 4, :, 1:32],
                                    H[96:112, cg2:cg2 + 4, :, 1:32])
                            for q in range(4):
                                cl = 4 * cg2 + q
                                chalf, c32 = cl // 32, cl % 32
                                yb = ypsp.tile([128, B, M], F32)
                                nc.tensor.matmul(
                                    yb[:],
                                    toepsb[:, cl:cl + 1, :].squeeze(),
                                    uT[:, :, :, cl:cl + 1].squeeze(),
                                    start=True, stop=False)
                                if q < 3:
                                    v1_l = v1sb[32 * q:32 * q + 16,
                                                cg2:cg2 + 1, 0:1, :].squeeze()
                                    h_r = H[32 * q:32 * q + 16,
                                            cg2:cg2 + 1, :, :].squeeze()
                                else:
                                    v1_l = v1sb[0:16, cg2:cg2 + 1,
                                                1:2, :].squeeze()
                                    h_r = H96[0:16, cg2:cg2 + 1, :, :].squeeze()
                                nc.tensor.matmul(
                                    yb[:], v1_l, h_r,
                                    start=False, stop=True)
                                sg = sgp.tile([T, 8, 2, M], YD)
                                nc.scalar.activation(sg[:], yb[:], AF.Gelu)
                                for bb in range(2):
                                    nc.sync.dma_start(
                                        ysend[2 * bb + chalf]
                                        [:, c32:c32 + 1, :, :].squeeze()
                                        .transpose([1, 0, 2]),
                                        sg[:, :, bb:bb + 1, :].squeeze())
                            if cg2 == 7:
                                nc.gpsimd.collective_compute(
                                    "AllToAll", ALU.bypass, replica_groups=RG,
                                    ins=[ysend[0].opt()], outs=[yrecv[0].opt()])
                            if cg2 == 15:
                                nc.gpsimd.collective_compute(
                                    "AllToAll", ALU.bypass, replica_groups=RG,
                                    ins=[ysend[1].opt()], outs=[yrecv[1].opt()])
                                nc.gpsimd.collective_compute(
                                    "AllToAll", ALU.bypass, replica_groups=RG,
                                    ins=[ysend[2].opt()], outs=[yrecv[2].opt()])
                                nc.gpsimd.collective_compute(
                                    "AllToAll", ALU.bypass, replica_groups=RG,
                                    ins=[ysend[3].opt()], outs=[yrecv[3].opt()])

            # ============== Phase 2: mix ================================
            with tc.tile_pool(name="ytp", bufs=1) as ytp, \
                 tc.tile_pool(name="zps", bufs=3, space="PSUM") as zpsp, \
                 tc.tile_pool(name="mxs", bufs=4) as mxsp, \
                 tc.tile_pool(name="m1p", bufs=1) as m1p:
                wm = m1p.tile([128, 4, 1024], YD)
                nc.scalar.dma_start(wm[:], d_wmix.transpose([1, 0, 2]))
                M1 = m1p.tile([128, 2, 4, 8], F32)
                yt = {}
                for bb in range(2):
                    yt[bb] = ytp.tile([128, 4, T, M], YD, name=f"yt{bb}")
                    for ctt in range(4):
                        chalf, sh = ctt // 2, ctt % 2
                        for si in range(4):
                            s = 4 * sh + si
                            eng = nc.scalar if bb == 0 else nc.sync
                            eng.dma_start(
                                yt[bb][32 * si:32 * si + 32,
                                       ctt:ctt + 1, :, :].squeeze(),
                                yrecv[2 * bb + chalf][s:s + 1].squeeze())

                    for pr in range(4):
                        for lg in range(8):
                            zg = zpsp.tile([128, 16, M], F32)
                            za = zpsp.tile([128, 16, M], F32)
                            for side, ztile in ((1, zg), (0, za)):
                                ob = pr + 4 * side
                                if Y8:
                                    for t in range(2):
                                        nc.tensor.matmul(
                                            ztile[:],
                                            wm[:, 2 * t:2 * t + 2,
                                               128 * ob:128 * ob + 128],
                                            yt[bb][:, 2 * t:2 * t + 2,
                                                   16 * lg:16 * lg + 16, :],
                                            start=(t == 0), stop=(t == 1),
                                            perf_mode=mybir.MatmulPerfMode
                                            .DoubleRow)
                                else:
                                    for ctt in range(4):
                                        nc.tensor.matmul(
                                            ztile[:],
                                            wm[:, ctt:ctt + 1,
                                               128 * ob:128 * ob + 128]
                                            .squeeze(),
                                            yt[bb][:, ctt:ctt + 1,
                                                   16 * lg:16 * lg + 16, :]
                                            .squeeze(),
                                            start=(ctt == 0), stop=(ctt == 3))
                            sgm = mxsp.tile([128, 16, M], F16, tag="sgm")
                            nc.scalar.activation(
                                sgm[:], zg[:], AF.Sigmoid,
                                bias=boutg_sb[:, pr:pr + 1])
                            scr = mxsp.tile([128, 16, M], F16, tag="scr")
                            nc.vector.scalar_tensor_tensor(
                                scr[:], za[:], bouta_sb[:, pr:pr + 1], sgm[:],
                                op0=ALU.add, op1=ALU.mult,
                                accum_out=M1[:, bb:bb + 1, pr:pr + 1,
                                             lg:lg + 1].squeeze().unsqueeze(1))

                # ---- decode ----
                with tc.tile_pool(name="dps", bufs=1, space="PSUM") as dpsp:
                    R1 = m1p.tile([128, 2, 4], F32)
                    nc.vector.reduce_sum(R1[:], M1[:], axis=mybir.AxisListType.X)
                    R2 = m1p.tile([128, 2, 4], F32)
                    nc.vector.tensor_mul(R2[:], R1[:], wdec_sb[:])
                    R3 = m1p.tile([128, 2], F32)
                    nc.vector.reduce_sum(R3[:], R2[:], axis=mybir.AxisListType.X)
                    ones = m1p.tile([128, 1], F32)
                    nc.vector.memset(ones[:], 1.0)
                    dp = dpsp.tile([1, 2], F32)
                    nc.tensor.matmul(dp[:], ones[:], R3[:], start=True, stop=True)
                    osb = m1p.tile([1, 2], F32)
                    nc.vector.tensor_scalar_add(osb[:], dp[:], bdec_sb[:, 0:1])
                    nc.sync.dma_start(d_out[:], osb[:])

            biasp_cm.__exit__(None, None, None)

    nc.compile()
    return nc


def _get_compiled():
    global _compiled
    if _compiled is None:
        _compiled = _build()
    return _compiled


def _run(inputs, trace=False, **kw):
    in_maps = _prep(inputs)
    nc = _get_compiled()
    return bass_utils.run_bass_kernel_spmd(
        nc, in_maps, core_ids=list(range(NCORES)), trace=trace, **kw)


def kernel(**inputs):
    inputs = {k: np.asarray(v) for k, v in inputs.items()}
    res = _run(inputs)
    out = np.empty((B, 1), np.float32)
    for cid in range(NCORES):
        out[2 * cid:2 * cid + 2, 0] = res.results[cid]["odec"][0, :]
    return out
